# revision 2
# baseline (speedup 1.0000x reference)
"""Expert-parallel DeepseekV2 MoE kernel for 8 Trainium2 NeuronCores, v2.

Strategy (vs the v1 baseline):
  - All weights and activations stored/streamed in bf16: halves HBM traffic
    (the dominant cost in this memory-regime problem). PSUM accumulation
    stays fp32; outputs are written bf16 and combined on host in fp32.
  - Token tiles of 128 (not 256): experts are decomposed into 2-tile slots
    (<=256 tokens) and 1-tile slots (<=128 tokens). Every core runs the
    identical SPMD schedule: N2 two-tile slots + N1 one-tile slots
    (for the typical routing: 3 + 2 = 8 token-tiles per core).
  - Gate/up matmuls keep the token tile STATIONARY (x^T tile [d, t]) and
    stream the weights as the moving operand: 1 LDWEIGHTS per (k-tile,
    token-tile) amortized over 1408 moving columns, instead of one per
    128x128 weight tile.
  - h is transposed on the PE (identity matmul) so the down matmul also
    runs with the token tile stationary (h^T [i, t]) and wd moving.
  - Shared MLP is tensor-parallel on the intermediate dim (352/core),
    same stationary-token structure; partials summed on host.
"""

import math

import numpy as np
import ml_dtypes

import concourse.bass as bass
import concourse.tile as tile
from concourse import bacc, mybir
from concourse.bass_utils import run_bass_kernel_spmd

# Problem shapes (hardcoded per the harness contract).
T, D = 1024, 2048
E, I = 32, 1408
TOPK = 6
N_GROUP, TOPK_GROUP = 8, 3
ROUTED_SCALE = 2.5
SHARED_I = 2 * I  # 2816

NCORES = 8
ISH = SHARED_I // NCORES   # 352 shared-intermediate per core
KT = D // 128              # 16 contraction tiles over D
IT = I // 128              # 11 intermediate tiles
ISC = 3                    # shared-intermediate chunks (128,128,96)
IS_SZ = [128, 128, ISH - 256]

F32 = mybir.dt.float32
BF16 = mybir.dt.bfloat16
SILU = mybir.ActivationFunctionType.Silu
BF = ml_dtypes.bfloat16

_PROGRAM_CACHE = {}


def _build_program(n2, n1):
    """n2 two-tile slots + n1 one-tile slots per core."""
    nc = bacc.Bacc("TRN2", target_bir_lowering=False, debug=False)

    slot_caps = [256] * n2 + [128] * n1

    xt = nc.dram_tensor("xt", [128, KT * T], BF16, kind="ExternalInput").ap()
    wsg = nc.dram_tensor("wsg", [128, KT * ISH], BF16, kind="ExternalInput").ap()
    wsu = nc.dram_tensor("wsu", [128, KT * ISH], BF16, kind="ExternalInput").ap()
    wsd = nc.dram_tensor("wsd", [128, ISC * D], BF16, kind="ExternalInput").ap()
    ident = nc.dram_tensor("ident", [128, 128], BF16, kind="ExternalInput").ap()
    ys = nc.dram_tensor("ys", [T // 128, 128, D], BF16, kind="ExternalOutput").ap()

    xg_d, wg_d, wu_d, wd_d, ye_d = [], [], [], [], []
    for s, cap in enumerate(slot_caps):
        ntt = cap // 128
        xg_d.append(nc.dram_tensor(f"xg{s}", [128, KT * cap], BF16,
                                   kind="ExternalInput").ap())
        wg_d.append(nc.dram_tensor(f"wg{s}", [4, 128, 4 * I], BF16,
                                   kind="ExternalInput").ap())
        wu_d.append(nc.dram_tensor(f"wu{s}", [4, 128, 4 * I], BF16,
                                   kind="ExternalInput").ap())
        wd_d.append(nc.dram_tensor(f"wd{s}", [128, IT * D], BF16,
                                   kind="ExternalInput").ap())
        ye_d.append(nc.dram_tensor(f"ye{s}", [ntt, 128, D], BF16,
                                   kind="ExternalOutput").ap())

    with tile.TileContext(nc) as tc, \
         tc.tile_pool(name="psum", bufs=8, space="PSUM") as psum, \
         tc.tile_pool(name="shres", bufs=1) as shres, \
         tc.tile_pool(name="hspool", bufs=3) as hspool, \
         tc.tile_pool(name="yspool", bufs=2) as yspool, \
         tc.tile_pool(name="xgpool", bufs=2) as xgpool, \
         tc.tile_pool(name="wpool", bufs=3) as wpool, \
         tc.tile_pool(name="wdpool", bufs=2) as wdpool, \
         tc.tile_pool(name="hgpool", bufs=3) as hgpool, \
         tc.tile_pool(name="hpool", bufs=3) as hpool, \
         tc.tile_pool(name="htpool", bufs=3) as htpool, \
         tc.tile_pool(name="ypool", bufs=3) as ypool:

        # ---- resident shared inputs ----
        xt_sb = shres.tile([128, KT * T], BF16, tag="xt")
        nc.sync.dma_start(out=xt_sb[:], in_=xt)
        wsg_sb = shres.tile([128, KT * ISH], BF16, tag="wsg")
        nc.sync.dma_start(out=wsg_sb[:], in_=wsg)
        wsu_sb = shres.tile([128, KT * ISH], BF16, tag="wsu")
        nc.sync.dma_start(out=wsu_sb[:], in_=wsu)
        wsd_sb = shres.tile([128, ISC * D], BF16, tag="wsd")
        nc.sync.dma_start(out=wsd_sb[:], in_=wsd)
        id_sb = shres.tile([128, 128], BF16, tag="ident")
        nc.sync.dma_start(out=id_sb[:], in_=ident)

        # hs^T resident: [is-part 128, chunk 3, tokens 1024] bf16
        hsT_sb = shres.tile([128, ISC, T], BF16, tag="hsT")

        # ---- shared m1: hs[t, is] = silu(x wsg) * (x wsu), by token pairs --
        for grp in range(4):          # groups of 2 token-tiles -> 4 psum banks
            tts = (2 * grp, 2 * grp + 1)
            psg = {tt: psum.tile([128, 512], F32, tag="ps", name=f"sg{tt}")
                   for tt in tts}
            psu = {tt: psum.tile([128, 512], F32, tag="ps", name=f"su{tt}")
                   for tt in tts}
            for kc in range(KT):
                for tt in tts:
                    lhs = xt_sb[:, kc * T + tt * 128: kc * T + (tt + 1) * 128]
                    nc.tensor.matmul(psg[tt][:, :ISH], lhs,
                                     wsg_sb[:, kc * ISH:(kc + 1) * ISH],
                                     start=(kc == 0), stop=(kc == KT - 1))
                    nc.tensor.matmul(psu[tt][:, :ISH], lhs,
                                     wsu_sb[:, kc * ISH:(kc + 1) * ISH],
                                     start=(kc == 0), stop=(kc == KT - 1))
            for tt in tts:
                hsg = hspool.tile([128, ISH], BF16, tag="hsg")
                nc.scalar.activation(hsg[:], psg[tt][:, :ISH], SILU)
                hs = hspool.tile([128, ISH], BF16, tag="hs")
                nc.vector.tensor_mul(hs[:], psu[tt][:, :ISH], hsg[:])
                # transpose hs -> hsT (3 chunks of is)
                for c in range(ISC):
                    sz = IS_SZ[c]
                    pst = psum.tile([128, 512], BF16, tag="ps",
                                    name=f"st{tt}_{c}")
                    nc.tensor.transpose(pst[:sz, :128],
                                        hs[:, c * 128:c * 128 + sz], id_sb[:])
                    nc.vector.tensor_copy(
                        hsT_sb[:sz, c, tt * 128:(tt + 1) * 128],
                        pst[:sz, :128])

        # ---- shared m2: ys[t, d] = hs @ wsd  (hsT stationary, wsd moving) --
        for tt in range(T // 128):
            psy = [psum.tile([128, 512], F32, tag="ps", name=f"sy{tt}_{q}")
                   for q in range(4)]
            for c in range(ISC):
                sz = IS_SZ[c]
                for q in range(4):
                    nc.tensor.matmul(
                        psy[q][:],
                        hsT_sb[:sz, c, tt * 128:(tt + 1) * 128],
                        wsd_sb[:sz, c * D + q * 512: c * D + (q + 1) * 512],
                        start=(c == 0), stop=(c == ISC - 1))
            ysb = yspool.tile([128, D], BF16, tag="ysb")
            for q in range(4):
                nc.vector.tensor_copy(ysb[:, q * 512:(q + 1) * 512], psy[q][:])
            nc.sync.dma_start(out=ys[tt], in_=ysb[:])

        # ---- routed expert slots ----
        for s, cap in enumerate(slot_caps):
            ntt = cap // 128
            xg_sb = xgpool.tile([128, KT * 256], BF16, tag="xg")
            nc.sync.dma_start(out=xg_sb[:, :KT * cap], in_=xg_d[s])

            def xg_lhs(kc, tt):
                return xg_sb[:, kc * cap + tt * 128: kc * cap + (tt + 1) * 128]

            # gate pass, then up pass (psum: 3 banks per token-tile)
            hgs = {}
            hss = {}
            for w_dram, is_gate in ((wg_d[s], True), (wu_d[s], False)):
                ps = {(tt, j): psum.tile([128, 512], F32, tag="ps",
                                         name=f"p{s}_{int(is_gate)}_{tt}_{j}")
                      for tt in range(ntt) for j in range(3)}
                for ch in range(4):
                    w_sb = wpool.tile([128, 4 * I], BF16, tag="wst")
                    nc.sync.dma_start(out=w_sb[:], in_=w_dram[ch])
                    for a in range(4):
                        kc = ch * 4 + a
                        for tt in range(ntt):
                            lhs = xg_lhs(kc, tt)
                            for j in range(3):
                                sz = 512 if j < 2 else I - 1024
                                nc.tensor.matmul(
                                    ps[(tt, j)][:, :sz], lhs,
                                    w_sb[:, a * I + j * 512:
                                         a * I + j * 512 + sz],
                                    start=(kc == 0), stop=(kc == KT - 1))
                for tt in range(ntt):
                    if is_gate:
                        hg = hgpool.tile([128, I], BF16, tag="hg")
                        for j in range(3):
                            sz = 512 if j < 2 else I - 1024
                            nc.scalar.activation(
                                hg[:, j * 512:j * 512 + sz],
                                ps[(tt, j)][:, :sz], SILU)
                        hgs[tt] = hg
                    else:
                        h = hpool.tile([128, I], BF16, tag="h")
                        for j in range(3):
                            sz = 512 if j < 2 else I - 1024
                            nc.vector.tensor_mul(
                                h[:, j * 512:j * 512 + sz],
                                ps[(tt, j)][:, :sz],
                                hgs[tt][:, j * 512:j * 512 + sz])
                        hss[tt] = h

            # transpose h -> h^T per token-tile
            hts = {}
            for tt in range(ntt):
                ht = htpool.tile([128, IT * 128], BF16, tag="ht")
                for ic in range(IT):
                    pst = psum.tile([128, 512], BF16, tag="ps",
                                    name=f"t{s}_{tt}_{ic}")
                    nc.tensor.transpose(
                        pst[:, :128],
                        hss[tt][:, ic * 128:(ic + 1) * 128], id_sb[:])
                    nc.vector.tensor_copy(
                        ht[:, ic * 128:(ic + 1) * 128], pst[:, :128])
                hts[tt] = ht

            # down pass: y[t, d] = h^T.T @ wd (h^T stationary, wd moving)
            psy = {(tt, q): psum.tile([128, 512], F32, tag="ps",
                                      name=f"y{s}_{tt}_{q}")
                   for tt in range(ntt) for q in range(4)}
            ichunks = [(0, 2), (2, 2), (4, 2), (6, 2), (8, 2), (10, 1)]
            for i0, cnt in ichunks:
                wd_sb = wdpool.tile([128, 2 * D], BF16, tag="wdst")
                nc.sync.dma_start(out=wd_sb[:, :cnt * D],
                                  in_=wd_d[s][:, i0 * D:(i0 + cnt) * D])
                for a in range(cnt):
                    i = i0 + a
                    for tt in range(ntt):
                        for q in range(4):
                            nc.tensor.matmul(
                                psy[(tt, q)][:],
                                hts[tt][:, i * 128:(i + 1) * 128],
                                wd_sb[:, a * D + q * 512:
                                      a * D + (q + 1) * 512],
                                start=(i == 0), stop=(i == IT - 1))
            for tt in range(ntt):
                ysb = ypool.tile([128, D], BF16, tag="ye_sb")
                for q in range(4):
                    nc.vector.tensor_copy(ysb[:, q * 512:(q + 1) * 512],
                                          psy[(tt, q)][:])
                nc.sync.dma_start(out=ye_d[s][tt], in_=ysb[:])

    nc.compile()
    return nc


def get_program(n2, n1):
    key = (n2, n1)
    if key not in _PROGRAM_CACHE:
        _PROGRAM_CACHE[key] = _build_program(n2, n1)
    return _PROGRAM_CACHE[key]


def _route_numpy(x, gate_w, bias):
    """Mirror reference.py's grouped top-k routing in fp32 numpy."""
    logits = x @ gate_w                                   # [T, E]
    scores = 1.0 / (1.0 + np.exp(-logits))
    sc = scores + bias[None, :]
    g = sc.reshape(-1, N_GROUP, E // N_GROUP)
    group_scores = np.sort(g, axis=-1)[..., -2:].sum(-1)  # [T, n_group]
    gidx = np.argsort(-group_scores, axis=-1, kind="stable")[:, :TOPK_GROUP]
    gmask = np.zeros((x.shape[0], N_GROUP), np.bool_)
    np.put_along_axis(gmask, gidx, True, axis=-1)
    emask = np.repeat(gmask, E // N_GROUP, axis=-1)       # [T, E]
    masked = np.where(emask, sc, -np.inf)
    topk_idx = np.argsort(-masked, axis=-1, kind="stable")[:, :TOPK]
    w = np.take_along_axis(scores, topk_idx, axis=-1)
    w = w / (w.sum(-1, keepdims=True) + 1e-20)
    return topk_idx, w


def _plan(topk_idx, topk_w):
    """Decompose expert token lists into 2-tile (<=256 tok) and 1-tile
    (<=128 tok) slots, pad/split so each count divides by NCORES, and deal
    round-robin. Returns (per_core 2-slots, per_core 1-slots)."""
    flat_e = topk_idx.ravel()
    flat_t = np.repeat(np.arange(topk_idx.shape[0]), TOPK)
    flat_w = (topk_w * ROUTED_SCALE).ravel().astype(np.float32)
    order = np.argsort(flat_e, kind="stable")
    sorted_t = flat_t[order]
    sorted_w = flat_w[order]
    counts = np.bincount(flat_e, minlength=E)
    offsets = np.concatenate([[0], np.cumsum(counts)])

    two_slots, one_slots = [], []   # (expert, tok_idx, weights)
    for e in range(E):
        toks = sorted_t[offsets[e]:offsets[e + 1]]
        ws = sorted_w[offsets[e]:offsets[e + 1]]
        n = len(toks)
        if n == 0:
            continue
        pos = 0
        while n - pos > 128:
            two_slots.append((e, toks[pos:pos + 256], ws[pos:pos + 256]))
            pos += 256
        if n - pos > 0:
            one_slots.append((e, toks[pos:], ws[pos:]))

    # fix divisibility: a = promote 1->2 (pad), b = split 2 -> two 1s,
    # e2/e1 = dummy empty slots (weights still streamed). cost ~ tiles
    # wasted * 3 + streams wasted * 2.
    best = None
    for a in range(9):
        for b in range(9):
            for e2 in range(8):
                for e1 in range(8):
                    if a > len(one_slots) or b > len(two_slots):
                        continue
                    t2 = len(two_slots) + a - b + e2
                    t1 = len(one_slots) - a + 2 * b + e1
                    if t2 % NCORES or t1 % NCORES or t2 + t1 == 0:
                        continue
                    cost = 3 * (a + 2 * e2 + e1) + 2 * (b + e2 + e1)
                    if best is None or cost < best[0]:
                        best = (cost, a, b, e2, e1)
    _, a, b, e2, e1 = best
    for _ in range(a):      # promote shortest 1-slots to (padded) 2-slots
        one_slots.sort(key=lambda s: len(s[1]))
        two_slots.append(one_slots.pop(0))
    for _ in range(b):      # split longest 2-slots into two 1-slots
        two_slots.sort(key=lambda s: len(s[1]))
        e, tk, ws = two_slots.pop()
        one_slots.append((e, tk[:128], ws[:128]))
        one_slots.append((e, tk[128:], ws[128:]))
    empty = (0, np.empty(0, np.int64), np.empty(0, np.float32))
    for _ in range(e2):
        two_slots.append(empty)
    for _ in range(e1):
        one_slots.append(empty)

    n2 = len(two_slots) // NCORES
    n1 = len(one_slots) // NCORES
    per_core = [[] for _ in range(NCORES)]
    for si, s in enumerate(two_slots):
        per_core[si % NCORES].append(s)
    for si, s in enumerate(one_slots):
        per_core[si % NCORES].append(s)
    return per_core, n2, n1


def _pack_k(a):
    """[D, m] -> [128, KT*m] with k-tile-major packing."""
    m = a.shape[1]
    return np.ascontiguousarray(
        a.reshape(KT, 128, m).transpose(1, 0, 2).reshape(128, KT * m))


def _pack_w_chunks(w):
    """[D, I] -> [4, 128, 4*I]: chunk c holds k-tiles 4c..4c+3 side by side."""
    return np.ascontiguousarray(
        w.reshape(4, 4, 128, I).transpose(0, 2, 1, 3).reshape(4, 128, 4 * I))


def build_in_maps(inputs):
    x = np.asarray(inputs["hidden_states"], np.float32)
    gate_w = np.asarray(inputs["gate_w"], np.float32)
    bias = np.asarray(inputs["e_score_correction_bias"], np.float32)
    w_gate = np.asarray(inputs["w_gate"], np.float32)
    w_up = np.asarray(inputs["w_up"], np.float32)
    w_down = np.asarray(inputs["w_down"], np.float32)
    ws_gate = np.asarray(inputs["ws_gate"], np.float32)
    ws_up = np.asarray(inputs["ws_up"], np.float32)
    ws_down = np.asarray(inputs["ws_down"], np.float32)

    topk_idx, topk_w = _route_numpy(x, gate_w, bias)
    per_core, n2, n1 = _plan(topk_idx, topk_w)

    xt_bf = np.ascontiguousarray(x.T.astype(BF))            # [D, T]
    xt_packed = _pack_k(xt_bf)
    wg_bf = w_gate.astype(BF)
    wu_bf = w_up.astype(BF)
    wd_bf = w_down.astype(BF)
    wg_chunks = {}
    wu_chunks = {}
    wd_tiles = {}

    def expert_w(e):
        if e not in wg_chunks:
            wg_chunks[e] = _pack_w_chunks(wg_bf[e])
            wu_chunks[e] = _pack_w_chunks(wu_bf[e])
            wd_tiles[e] = np.ascontiguousarray(
                wd_bf[e].reshape(IT, 128, D).transpose(1, 0, 2)
                .reshape(128, IT * D))
        return wg_chunks[e], wu_chunks[e], wd_tiles[e]

    wsg_bf = ws_gate.astype(BF)
    wsu_bf = ws_up.astype(BF)
    wsd_bf = ws_down.astype(BF)
    identity = np.eye(128, dtype=BF)

    in_maps = []
    for c in range(NCORES):
        m = {
            "xt": xt_packed,
            "wsg": _pack_k(wsg_bf[:, c * ISH:(c + 1) * ISH]),
            "wsu": _pack_k(wsu_bf[:, c * ISH:(c + 1) * ISH]),
            "ident": identity,
        }
        wsd_sl = np.zeros((ISC * 128, D), BF)
        wsd_sl[:ISH] = wsd_bf[c * ISH:(c + 1) * ISH]
        m["wsd"] = np.ascontiguousarray(
            wsd_sl.reshape(ISC, 128, D).transpose(1, 0, 2).reshape(128, ISC * D))
        for s, (e, idx, _) in enumerate(per_core[c]):
            cap = 256 if s < n2 else 128
            xg = np.zeros((D, cap), BF)
            if len(idx):
                xg[:, :len(idx)] = xt_bf[:, idx]
            m[f"xg{s}"] = _pack_k(xg)
            wgc, wuc, wdt = expert_w(e)
            m[f"wg{s}"] = wgc
            m[f"wu{s}"] = wuc
            m[f"wd{s}"] = wdt
        in_maps.append(m)
    return in_maps, per_core, n2, n1


def kernel(**inputs):
    in_maps, per_core, n2, n1 = build_in_maps(inputs)
    nc = get_program(n2, n1)
    res = run_bass_kernel_spmd(nc, in_maps, core_ids=list(range(NCORES)))

    out = np.zeros((T, D), np.float32)
    for c in range(NCORES):
        r = res.results[c]
        out += r["ys"].reshape(T, D).astype(np.float32)
        for s, (e, idx, wv) in enumerate(per_core[c]):
            if not len(idx):
                continue
            cap = 256 if s < n2 else 128
            y = r[f"ye{s}"].reshape(cap, D)[:len(idx)].astype(np.float32)
            out[idx] += wv[:, None] * y
    return out.astype(np.float32)


# revision 3
# speedup vs baseline: 1.0273x; 1.0273x over previous
"""Expert-parallel DeepseekV2 MoE kernel for 8 Trainium2 NeuronCores, v4.

vs v3:
  - ALL inputs in one [128, N] bf16 tensor (per-iteration overhead through
    this exec path is ~29us per argument, so argument count is minimized).
  - Shared m1 computes hs^T directly (wsg/wsu tiles stationary, x^T moving)
    instead of m1-then-PE-transpose: fewer PE ops, fewer DVE copies.

Layout of din columns:
  [ xt_packed (KT*T) | xg slot 0..n (KT*cap each) | ident (128)
  | wsg (KT*ISH) | wsu (KT*ISH) | wsd (ISC*D)
  | slot 0: wg 16*I | wu 16*I | wd IT*D | slot 1: ... ]
Output rows: [ys tile 0..7 | ye slot tiles in order].
"""

import numpy as np
import ml_dtypes

import concourse.bass as bass
import concourse.tile as tile
from concourse import bacc, mybir
from concourse.bass_utils import run_bass_kernel_spmd

T, D = 1024, 2048
E, I = 32, 1408
TOPK = 6
N_GROUP, TOPK_GROUP = 8, 3
ROUTED_SCALE = 2.5
SHARED_I = 2 * I

NCORES = 8
ISH = SHARED_I // NCORES   # 352
KT = D // 128              # 16
IT = I // 128              # 11
ISC = 3
IS_SZ = [128, 128, ISH - 256]
WSLOT = 16 * I + 16 * I + IT * D   # 67584 cols per routed slot

F32 = mybir.dt.float32
BF16 = mybir.dt.bfloat16
SILU = mybir.ActivationFunctionType.Silu
BF = ml_dtypes.bfloat16

_PROGRAM_CACHE = {}


def _col_layout(slot_caps):
    off = {}
    o = 0
    off["xt"] = o; o += KT * T
    off["xg"] = []
    for c in slot_caps:
        off["xg"].append(o); o += KT * c
    off["ident"] = o; o += 128
    off["wsg"] = o; o += KT * ISH
    off["wsu"] = o; o += KT * ISH
    off["wsd"] = o; o += ISC * D
    off["wr"] = []
    for _ in slot_caps:
        off["wr"].append(o); o += WSLOT
    off["total"] = o
    return off


def _build_program(n2, n1):
    nc = bacc.Bacc("TRN2", target_bir_lowering=False, debug=False)

    slot_caps = [256] * n2 + [128] * n1
    ntt_total = sum(c // 128 for c in slot_caps)
    off = _col_layout(slot_caps)

    din = nc.dram_tensor("din", [128, off["total"]], BF16,
                         kind="ExternalInput").ap()
    yo = nc.dram_tensor("yo", [8 + ntt_total, 128, D], BF16,
                        kind="ExternalOutput").ap()

    with tile.TileContext(nc) as tc, \
         tc.tile_pool(name="psum", bufs=8, space="PSUM") as psum, \
         tc.tile_pool(name="shres", bufs=1) as shres, \
         tc.tile_pool(name="hspool", bufs=3) as hspool, \
         tc.tile_pool(name="yspool", bufs=2) as yspool, \
         tc.tile_pool(name="xgpool", bufs=2) as xgpool, \
         tc.tile_pool(name="wpool", bufs=3) as wpool, \
         tc.tile_pool(name="wdpool", bufs=2) as wdpool, \
         tc.tile_pool(name="hgpool", bufs=3) as hgpool, \
         tc.tile_pool(name="hpool", bufs=3) as hpool, \
         tc.tile_pool(name="htpool", bufs=3) as htpool, \
         tc.tile_pool(name="ypool", bufs=3) as ypool:

        xt_sb = shres.tile([128, KT * T], BF16, tag="xt")
        nc.sync.dma_start(out=xt_sb[:], in_=din[:, :KT * T])
        wsg_sb = shres.tile([128, KT * ISH], BF16, tag="wsg")
        nc.sync.dma_start(out=wsg_sb[:],
                          in_=din[:, off["wsg"]:off["wsg"] + KT * ISH])
        wsu_sb = shres.tile([128, KT * ISH], BF16, tag="wsu")
        nc.sync.dma_start(out=wsu_sb[:],
                          in_=din[:, off["wsu"]:off["wsu"] + KT * ISH])
        wsd_sb = shres.tile([128, ISC * D], BF16, tag="wsd")
        nc.sync.dma_start(out=wsd_sb[:],
                          in_=din[:, off["wsd"]:off["wsd"] + ISC * D])
        id_sb = shres.tile([128, 128], BF16, tag="ident")
        nc.sync.dma_start(out=id_sb[:],
                          in_=din[:, off["ident"]:off["ident"] + 128])

        hsT_sb = shres.tile([128, ISC, T], BF16, tag="hsT")

        # ---- shared m1: hs^T[is, t] directly (wsg/wsu stationary) ----
        for half in range(2):
            tsl = slice(half * 512, (half + 1) * 512)
            pg = {c: psum.tile([128, 512], F32, tag="ps", name=f"sg{half}_{c}")
                  for c in range(ISC)}
            pu = {c: psum.tile([128, 512], F32, tag="ps", name=f"su{half}_{c}")
                  for c in range(ISC)}
            for kc in range(KT):
                xmov = xt_sb[:, kc * T + half * 512: kc * T + (half + 1) * 512]
                for c in range(ISC):
                    sz = IS_SZ[c]
                    nc.tensor.matmul(
                        pg[c][:sz, :], wsg_sb[:, kc * ISH + c * 128:
                                              kc * ISH + c * 128 + sz],
                        xmov, start=(kc == 0), stop=(kc == KT - 1))
                    nc.tensor.matmul(
                        pu[c][:sz, :], wsu_sb[:, kc * ISH + c * 128:
                                              kc * ISH + c * 128 + sz],
                        xmov, start=(kc == 0), stop=(kc == KT - 1))
            for c in range(ISC):
                sz = IS_SZ[c]
                hsg = hspool.tile([128, 512], BF16, tag="hsg")
                nc.scalar.activation(hsg[:sz, :], pg[c][:sz, :], SILU)
                nc.vector.tensor_mul(hsT_sb[:sz, c, tsl],
                                     pu[c][:sz, :], hsg[:sz, :])

        # ---- shared m2: ys[t, d] = hs @ wsd (hsT stationary, wsd moving) --
        for tt in range(T // 128):
            psy = [psum.tile([128, 512], F32, tag="ps", name=f"sy{tt}_{q}")
                   for q in range(4)]
            for c in range(ISC):
                sz = IS_SZ[c]
                for q in range(4):
                    nc.tensor.matmul(
                        psy[q][:],
                        hsT_sb[:sz, c, tt * 128:(tt + 1) * 128],
                        wsd_sb[:sz, c * D + q * 512: c * D + (q + 1) * 512],
                        start=(c == 0), stop=(c == ISC - 1))
            ysb = yspool.tile([128, D], BF16, tag="ysb")
            for q in range(4):
                nc.vector.tensor_copy(ysb[:, q * 512:(q + 1) * 512], psy[q][:])
            nc.sync.dma_start(out=yo[tt], in_=ysb[:])

        # ---- routed slots ----
        yo_row = 8
        for s, cap in enumerate(slot_caps):
            ntt = cap // 128
            xgo = off["xg"][s]
            soff = off["wr"][s]
            xg_sb = xgpool.tile([128, KT * 256], BF16, tag="xg")
            nc.sync.dma_start(out=xg_sb[:, :KT * cap],
                              in_=din[:, xgo:xgo + KT * cap])

            def xg_lhs(kc, tt):
                return xg_sb[:, kc * cap + tt * 128: kc * cap + (tt + 1) * 128]

            hgs = {}
            hss = {}
            for mi, is_gate in ((0, True), (1, False)):
                moff = soff + mi * 16 * I
                ps = {(tt, j): psum.tile([128, 512], F32, tag="ps",
                                         name=f"p{s}_{mi}_{tt}_{j}")
                      for tt in range(ntt) for j in range(3)}
                for ch in range(4):
                    w_sb = wpool.tile([128, 4 * I], BF16, tag="wst")
                    nc.sync.dma_start(
                        out=w_sb[:],
                        in_=din[:, moff + ch * 4 * I: moff + (ch + 1) * 4 * I])
                    for a in range(4):
                        kc = ch * 4 + a
                        for tt in range(ntt):
                            lhs = xg_lhs(kc, tt)
                            for j in range(3):
                                sz = 512 if j < 2 else I - 1024
                                nc.tensor.matmul(
                                    ps[(tt, j)][:, :sz], lhs,
                                    w_sb[:, a * I + j * 512:
                                         a * I + j * 512 + sz],
                                    start=(kc == 0), stop=(kc == KT - 1))
                for tt in range(ntt):
                    if is_gate:
                        hg = hgpool.tile([128, I], BF16, tag="hg")
                        for j in range(3):
                            sz = 512 if j < 2 else I - 1024
                            nc.scalar.activation(
                                hg[:, j * 512:j * 512 + sz],
                                ps[(tt, j)][:, :sz], SILU)
                        hgs[tt] = hg
                    else:
                        h = hpool.tile([128, I], BF16, tag="h")
                        for j in range(3):
                            sz = 512 if j < 2 else I - 1024
                            nc.vector.tensor_mul(
                                h[:, j * 512:j * 512 + sz],
                                ps[(tt, j)][:, :sz],
                                hgs[tt][:, j * 512:j * 512 + sz])
                        hss[tt] = h

            hts = {}
            for tt in range(ntt):
                ht = htpool.tile([128, IT * 128], BF16, tag="ht")
                for ic in range(IT):
                    pst = psum.tile([128, 512], BF16, tag="ps",
                                    name=f"t{s}_{tt}_{ic}")
                    nc.tensor.transpose(
                        pst[:, :128],
                        hss[tt][:, ic * 128:(ic + 1) * 128], id_sb[:])
                    nc.vector.tensor_copy(
                        ht[:, ic * 128:(ic + 1) * 128], pst[:, :128])
                hts[tt] = ht

            wdoff = soff + 2 * 16 * I
            psy = {(tt, q): psum.tile([128, 512], F32, tag="ps",
                                      name=f"y{s}_{tt}_{q}")
                   for tt in range(ntt) for q in range(4)}
            ichunks = [(0, 2), (2, 2), (4, 2), (6, 2), (8, 2), (10, 1)]
            for i0, cnt in ichunks:
                wd_sb = wdpool.tile([128, 2 * D], BF16, tag="wdst")
                nc.sync.dma_start(
                    out=wd_sb[:, :cnt * D],
                    in_=din[:, wdoff + i0 * D: wdoff + (i0 + cnt) * D])
                for a in range(cnt):
                    i = i0 + a
                    for tt in range(ntt):
                        for q in range(4):
                            nc.tensor.matmul(
                                psy[(tt, q)][:],
                                hts[tt][:, i * 128:(i + 1) * 128],
                                wd_sb[:, a * D + q * 512:
                                      a * D + (q + 1) * 512],
                                start=(i == 0), stop=(i == IT - 1))
            for tt in range(ntt):
                ysb = ypool.tile([128, D], BF16, tag="ye_sb")
                for q in range(4):
                    nc.vector.tensor_copy(ysb[:, q * 512:(q + 1) * 512],
                                          psy[(tt, q)][:])
                nc.sync.dma_start(out=yo[yo_row], in_=ysb[:])
                yo_row += 1

    nc.compile()
    return nc


def get_program(n2, n1):
    key = (n2, n1)
    if key not in _PROGRAM_CACHE:
        _PROGRAM_CACHE[key] = _build_program(n2, n1)
    return _PROGRAM_CACHE[key]


def _route_numpy(x, gate_w, bias):
    logits = x @ gate_w
    scores = 1.0 / (1.0 + np.exp(-logits))
    sc = scores + bias[None, :]
    g = sc.reshape(-1, N_GROUP, E // N_GROUP)
    group_scores = np.sort(g, axis=-1)[..., -2:].sum(-1)
    gidx = np.argsort(-group_scores, axis=-1, kind="stable")[:, :TOPK_GROUP]
    gmask = np.zeros((x.shape[0], N_GROUP), np.bool_)
    np.put_along_axis(gmask, gidx, True, axis=-1)
    emask = np.repeat(gmask, E // N_GROUP, axis=-1)
    masked = np.where(emask, sc, -np.inf)
    topk_idx = np.argsort(-masked, axis=-1, kind="stable")[:, :TOPK]
    w = np.take_along_axis(scores, topk_idx, axis=-1)
    w = w / (w.sum(-1, keepdims=True) + 1e-20)
    return topk_idx, w


def _plan(topk_idx, topk_w):
    flat_e = topk_idx.ravel()
    flat_t = np.repeat(np.arange(topk_idx.shape[0]), TOPK)
    flat_w = (topk_w * ROUTED_SCALE).ravel().astype(np.float32)
    order = np.argsort(flat_e, kind="stable")
    sorted_t = flat_t[order]
    sorted_w = flat_w[order]
    counts = np.bincount(flat_e, minlength=E)
    offsets = np.concatenate([[0], np.cumsum(counts)])

    two_slots, one_slots = [], []
    for e in range(E):
        toks = sorted_t[offsets[e]:offsets[e + 1]]
        ws_ = sorted_w[offsets[e]:offsets[e + 1]]
        n = len(toks)
        if n == 0:
            continue
        pos = 0
        while n - pos > 128:
            two_slots.append((e, toks[pos:pos + 256], ws_[pos:pos + 256]))
            pos += 256
        if n - pos > 0:
            one_slots.append((e, toks[pos:], ws_[pos:]))

    best = None
    for a in range(9):
        for b in range(9):
            for e2 in range(8):
                for e1 in range(8):
                    if a > len(one_slots) or b > len(two_slots):
                        continue
                    t2 = len(two_slots) + a - b + e2
                    t1 = len(one_slots) - a + 2 * b + e1
                    if t2 % NCORES or t1 % NCORES or t2 + t1 == 0:
                        continue
                    cost = 3 * (a + 2 * e2 + e1) + 2 * (b + e2 + e1)
                    if best is None or cost < best[0]:
                        best = (cost, a, b, e2, e1)
    _, a, b, e2, e1 = best
    for _ in range(a):
        one_slots.sort(key=lambda s: len(s[1]))
        two_slots.append(one_slots.pop(0))
    for _ in range(b):
        two_slots.sort(key=lambda s: len(s[1]))
        e, tk, ws_ = two_slots.pop()
        one_slots.append((e, tk[:128], ws_[:128]))
        one_slots.append((e, tk[128:], ws_[128:]))
    empty = (0, np.empty(0, np.int64), np.empty(0, np.float32))
    for _ in range(e2):
        two_slots.append(empty)
    for _ in range(e1):
        one_slots.append(empty)

    n2 = len(two_slots) // NCORES
    n1 = len(one_slots) // NCORES
    per_core = [[] for _ in range(NCORES)]
    for si, s in enumerate(two_slots):
        per_core[si % NCORES].append(s)
    for si, s in enumerate(one_slots):
        per_core[si % NCORES].append(s)
    return per_core, n2, n1


def _pack_k(a):
    m = a.shape[1]
    return np.ascontiguousarray(
        a.reshape(KT, 128, m).transpose(1, 0, 2).reshape(128, KT * m))


def _pack_w_chunks(w):
    """[D, I] -> [128, 16*I]: 4-ktile chunks side by side."""
    return np.ascontiguousarray(
        w.reshape(4, 4, 128, I).transpose(2, 0, 1, 3).reshape(128, 16 * I))


def build_in_maps(inputs):
    x = np.asarray(inputs["hidden_states"], np.float32)
    gate_w = np.asarray(inputs["gate_w"], np.float32)
    bias = np.asarray(inputs["e_score_correction_bias"], np.float32)
    w_gate = np.asarray(inputs["w_gate"], np.float32)
    w_up = np.asarray(inputs["w_up"], np.float32)
    w_down = np.asarray(inputs["w_down"], np.float32)
    ws_gate = np.asarray(inputs["ws_gate"], np.float32)
    ws_up = np.asarray(inputs["ws_up"], np.float32)
    ws_down = np.asarray(inputs["ws_down"], np.float32)

    topk_idx, topk_w = _route_numpy(x, gate_w, bias)
    per_core, n2, n1 = _plan(topk_idx, topk_w)
    slot_caps = [256] * n2 + [128] * n1
    off = _col_layout(slot_caps)

    xt_bf = np.ascontiguousarray(x.T.astype(BF))
    xt_packed = _pack_k(xt_bf)
    wg_bf = w_gate.astype(BF)
    wu_bf = w_up.astype(BF)
    wd_bf = w_down.astype(BF)
    wcache = {}

    def expert_w(e):
        if e not in wcache:
            wcache[e] = np.concatenate([
                _pack_w_chunks(wg_bf[e]),
                _pack_w_chunks(wu_bf[e]),
                wd_bf[e].reshape(IT, 128, D).transpose(1, 0, 2)
                .reshape(128, IT * D)], axis=1)
        return wcache[e]

    wsg_bf = ws_gate.astype(BF)
    wsu_bf = ws_up.astype(BF)
    wsd_bf = ws_down.astype(BF)
    identity = np.eye(128, dtype=BF)

    in_maps = []
    for c in range(NCORES):
        wsd_sl = np.zeros((ISC * 128, D), BF)
        wsd_sl[:ISH] = wsd_bf[c * ISH:(c + 1) * ISH]
        parts = [xt_packed]
        for s, (e, idx, _) in enumerate(per_core[c]):
            cap = slot_caps[s]
            xg = np.zeros((D, cap), BF)
            if len(idx):
                xg[:, :len(idx)] = xt_bf[:, idx]
            parts.append(_pack_k(xg))
        parts.append(identity)
        parts.append(_pack_k(wsg_bf[:, c * ISH:(c + 1) * ISH]))
        parts.append(_pack_k(wsu_bf[:, c * ISH:(c + 1) * ISH]))
        parts.append(wsd_sl.reshape(ISC, 128, D).transpose(1, 0, 2)
                     .reshape(128, ISC * D))
        for s, (e, idx, _) in enumerate(per_core[c]):
            parts.append(expert_w(e))
        din = np.ascontiguousarray(np.concatenate(parts, axis=1))
        assert din.shape[1] == off["total"]
        in_maps.append({"din": din})
    return in_maps, per_core, n2, n1


def kernel(**inputs):
    in_maps, per_core, n2, n1 = build_in_maps(inputs)
    nc = get_program(n2, n1)
    res = run_bass_kernel_spmd(nc, in_maps, core_ids=list(range(NCORES)))

    slot_caps = [256] * n2 + [128] * n1
    out = np.zeros((T, D), np.float32)
    for c in range(NCORES):
        r = res.results[c]["yo"].astype(np.float32)
        out += r[:8].reshape(T, D)
        row = 8
        for s, (e, idx, wv) in enumerate(per_core[c]):
            cap = slot_caps[s]
            ntt = cap // 128
            y = r[row:row + ntt].reshape(cap, D)
            row += ntt
            if len(idx):
                out[idx] += wv[:, None] * y[:len(idx)]
    return out.astype(np.float32)


# revision 4
# speedup vs baseline: 2.7925x; 2.7184x over previous
"""Expert-parallel DeepseekV2 MoE kernel for 8 Trainium2 NeuronCores, v5.

vs v3:
  - ALL inputs in one [128, N] bf16 tensor (per-iteration overhead through
    this exec path is ~29us per argument, so argument count is minimized).
  - Shared m1 computes hs^T directly (wsg/wsu tiles stationary, x^T moving)
    instead of m1-then-PE-transpose: fewer PE ops, fewer DVE copies.

Layout of din columns:
  [ xt_packed (KT*T) | xg slot 0..n (KT*cap each) | ident (128)
  | wsg (KT*ISH) | wsu (KT*ISH) | wsd (ISC*D)
  | slot 0: wg 16*I | wu 16*I | wd IT*D | slot 1: ... ]
Output rows: [ys tile 0..7 | ye slot tiles in order].
"""

import numpy as np
import ml_dtypes

import concourse.bass as bass
import concourse.tile as tile
from concourse import bacc, mybir
from concourse.bass_utils import run_bass_kernel_spmd

T, D = 1024, 2048
E, I = 32, 1408
TOPK = 6
N_GROUP, TOPK_GROUP = 8, 3
ROUTED_SCALE = 2.5
SHARED_I = 2 * I

NCORES = 8
ISH = SHARED_I // NCORES   # 352
KT = D // 128              # 16
IT = I // 128              # 11
ISC = 3
IS_SZ = [128, 128, ISH - 256]
WSLOT = 16 * I + 16 * I + IT * D   # 67584 cols per routed slot

F32 = mybir.dt.float32
BF16 = mybir.dt.bfloat16
SILU = mybir.ActivationFunctionType.Silu
BF = ml_dtypes.bfloat16

_PROGRAM_CACHE = {}


def _col_layout(slot_caps):
    off = {}
    o = 0
    off["xt"] = o; o += KT * T
    off["xg"] = []
    for c in slot_caps:
        off["xg"].append(o); o += KT * c
    off["ident"] = o; o += 128
    off["wsg"] = o; o += KT * ISH
    off["wsu"] = o; o += KT * ISH
    off["wsd"] = o; o += ISC * D
    off["wr"] = []
    for _ in slot_caps:
        off["wr"].append(o); o += WSLOT
    off["total"] = o
    return off


def _build_program(n2, n1):
    nc = bacc.Bacc("TRN2", target_bir_lowering=False, debug=False)

    slot_caps = [256] * n2 + [128] * n1
    ntt_total = sum(c // 128 for c in slot_caps)
    off = _col_layout(slot_caps)

    din = nc.dram_tensor("din", [128, off["total"]], BF16,
                         kind="ExternalInput").ap()
    yo = nc.dram_tensor("yo", [8 + ntt_total, 128, D], BF16,
                        kind="ExternalOutput").ap()

    with tile.TileContext(nc) as tc, \
         tc.tile_pool(name="psum", bufs=8, space="PSUM") as psum, \
         tc.tile_pool(name="shres", bufs=1) as shres, \
         tc.tile_pool(name="hspool", bufs=3) as hspool, \
         tc.tile_pool(name="yspool", bufs=2) as yspool, \
         tc.tile_pool(name="xgpool", bufs=2) as xgpool, \
         tc.tile_pool(name="wpool", bufs=4) as wpool, \
         tc.tile_pool(name="wdpool", bufs=2) as wdpool, \
         tc.tile_pool(name="hgpool", bufs=3) as hgpool, \
         tc.tile_pool(name="hpool", bufs=3) as hpool, \
         tc.tile_pool(name="htpool", bufs=3) as htpool, \
         tc.tile_pool(name="ypool", bufs=3) as ypool:

        xt_sb = shres.tile([128, KT * T], BF16, tag="xt")
        for piece in range(4):
            sl = slice(piece * 4 * T, (piece + 1) * 4 * T)
            nc.sync.dma_start(out=xt_sb[:, sl], in_=din[:, sl])
        wsg_sb = shres.tile([128, KT * ISH], BF16, tag="wsg")
        nc.sync.dma_start(out=wsg_sb[:],
                          in_=din[:, off["wsg"]:off["wsg"] + KT * ISH])
        wsu_sb = shres.tile([128, KT * ISH], BF16, tag="wsu")
        nc.sync.dma_start(out=wsu_sb[:],
                          in_=din[:, off["wsu"]:off["wsu"] + KT * ISH])
        wsd_sb = shres.tile([128, ISC * D], BF16, tag="wsd")
        nc.sync.dma_start(out=wsd_sb[:],
                          in_=din[:, off["wsd"]:off["wsd"] + ISC * D])
        id_sb = shres.tile([128, 128], BF16, tag="ident")
        nc.sync.dma_start(out=id_sb[:],
                          in_=din[:, off["ident"]:off["ident"] + 128])

        hsT_sb = shres.tile([128, ISC, T], BF16, tag="hsT")

        # ---- shared m1: hs^T[is, t] directly (wsg/wsu stationary) ----
        for half in range(2):
            tsl = slice(half * 512, (half + 1) * 512)
            pg = {c: psum.tile([128, 512], F32, tag="ps", name=f"sg{half}_{c}")
                  for c in range(ISC)}
            pu = {c: psum.tile([128, 512], F32, tag="ps", name=f"su{half}_{c}")
                  for c in range(ISC)}
            for kc in range(KT):
                xmov = xt_sb[:, kc * T + half * 512: kc * T + (half + 1) * 512]
                for c in range(ISC):
                    sz = IS_SZ[c]
                    nc.tensor.matmul(
                        pg[c][:sz, :], wsg_sb[:, kc * ISH + c * 128:
                                              kc * ISH + c * 128 + sz],
                        xmov, start=(kc == 0), stop=(kc == KT - 1))
                    nc.tensor.matmul(
                        pu[c][:sz, :], wsu_sb[:, kc * ISH + c * 128:
                                              kc * ISH + c * 128 + sz],
                        xmov, start=(kc == 0), stop=(kc == KT - 1))
            for c in range(ISC):
                sz = IS_SZ[c]
                hsg = hspool.tile([128, 512], BF16, tag="hsg")
                nc.scalar.activation(hsg[:sz, :], pg[c][:sz, :], SILU)
                nc.vector.tensor_mul(hsT_sb[:sz, c, tsl],
                                     pu[c][:sz, :], hsg[:sz, :])

        # ---- shared m2: ys[t, d] = hs @ wsd (hsT stationary, wsd moving) --
        for tt in range(T // 128):
            psy = [psum.tile([128, 512], F32, tag="ps", name=f"sy{tt}_{q}")
                   for q in range(4)]
            for c in range(ISC):
                sz = IS_SZ[c]
                for q in range(4):
                    nc.tensor.matmul(
                        psy[q][:],
                        hsT_sb[:sz, c, tt * 128:(tt + 1) * 128],
                        wsd_sb[:sz, c * D + q * 512: c * D + (q + 1) * 512],
                        start=(c == 0), stop=(c == ISC - 1))
            ysb = yspool.tile([128, D], BF16, tag="ysb")
            for q in range(4):
                nc.vector.tensor_copy(ysb[:, q * 512:(q + 1) * 512], psy[q][:])
            nc.sync.dma_start(out=yo[tt], in_=ysb[:])

        # ---- routed slots ----
        yo_row = 8
        for s, cap in enumerate(slot_caps):
            ntt = cap // 128
            xgo = off["xg"][s]
            soff = off["wr"][s]
            xg_sb = xgpool.tile([128, KT * 256], BF16, tag="xg")
            nc.sync.dma_start(out=xg_sb[:, :KT * cap],
                              in_=din[:, xgo:xgo + KT * cap])

            def xg_lhs(kc, tt):
                return xg_sb[:, kc * cap + tt * 128: kc * cap + (tt + 1) * 128]

            hgs = {}
            hss = {}
            for mi, is_gate in ((0, True), (1, False)):
                moff = soff + mi * 16 * I
                ps = {(tt, j): psum.tile([128, 512], F32, tag="ps",
                                         name=f"p{s}_{mi}_{tt}_{j}")
                      for tt in range(ntt) for j in range(3)}
                for ch in range(4):
                    w_sb = wpool.tile([128, 4 * I], BF16, tag="wst")
                    nc.sync.dma_start(
                        out=w_sb[:],
                        in_=din[:, moff + ch * 4 * I: moff + (ch + 1) * 4 * I])
                    for a in range(4):
                        kc = ch * 4 + a
                        for tt in range(ntt):
                            lhs = xg_lhs(kc, tt)
                            for j in range(3):
                                sz = 512 if j < 2 else I - 1024
                                nc.tensor.matmul(
                                    ps[(tt, j)][:, :sz], lhs,
                                    w_sb[:, a * I + j * 512:
                                         a * I + j * 512 + sz],
                                    start=(kc == 0), stop=(kc == KT - 1))
                for tt in range(ntt):
                    if is_gate:
                        hg = hgpool.tile([128, I], BF16, tag="hg")
                        for j in range(3):
                            sz = 512 if j < 2 else I - 1024
                            nc.scalar.activation(
                                hg[:, j * 512:j * 512 + sz],
                                ps[(tt, j)][:, :sz], SILU)
                        hgs[tt] = hg
                    else:
                        h = hpool.tile([128, I], BF16, tag="h")
                        for j in range(3):
                            sz = 512 if j < 2 else I - 1024
                            nc.vector.tensor_mul(
                                h[:, j * 512:j * 512 + sz],
                                ps[(tt, j)][:, :sz],
                                hgs[tt][:, j * 512:j * 512 + sz])
                        hss[tt] = h

            hts = {}
            for tt in range(ntt):
                ht = htpool.tile([128, IT * 128], BF16, tag="ht")
                for g0, gcnt in ((0, 4), (4, 4), (8, 3)):
                    pst = psum.tile([128, 512], BF16, tag="ps",
                                    name=f"t{s}_{tt}_{g0}")
                    for k in range(gcnt):
                        ic = g0 + k
                        nc.tensor.transpose(
                            pst[:, k * 128:(k + 1) * 128],
                            hss[tt][:, ic * 128:(ic + 1) * 128], id_sb[:])
                    nc.vector.tensor_copy(
                        ht[:, g0 * 128:(g0 + gcnt) * 128],
                        pst[:, :gcnt * 128])
                hts[tt] = ht

            wdoff = soff + 2 * 16 * I
            psy = {(tt, q): psum.tile([128, 512], F32, tag="ps",
                                      name=f"y{s}_{tt}_{q}")
                   for tt in range(ntt) for q in range(4)}
            ichunks = [(0, 2), (2, 2), (4, 2), (6, 2), (8, 2), (10, 1)]
            for i0, cnt in ichunks:
                wd_sb = wdpool.tile([128, 2 * D], BF16, tag="wdst")
                nc.sync.dma_start(
                    out=wd_sb[:, :cnt * D],
                    in_=din[:, wdoff + i0 * D: wdoff + (i0 + cnt) * D])
                for a in range(cnt):
                    i = i0 + a
                    for tt in range(ntt):
                        for q in range(4):
                            nc.tensor.matmul(
                                psy[(tt, q)][:],
                                hts[tt][:, i * 128:(i + 1) * 128],
                                wd_sb[:, a * D + q * 512:
                                      a * D + (q + 1) * 512],
                                start=(i == 0), stop=(i == IT - 1))
            for tt in range(ntt):
                ysb = ypool.tile([128, D], BF16, tag="ye_sb")
                for q in range(4):
                    nc.vector.tensor_copy(ysb[:, q * 512:(q + 1) * 512],
                                          psy[(tt, q)][:])
                nc.sync.dma_start(out=yo[yo_row], in_=ysb[:])
                yo_row += 1

    nc.compile()
    return nc


def get_program(n2, n1):
    key = (n2, n1)
    if key not in _PROGRAM_CACHE:
        _PROGRAM_CACHE[key] = _build_program(n2, n1)
    return _PROGRAM_CACHE[key]


def _route_numpy(x, gate_w, bias):
    logits = x @ gate_w
    scores = 1.0 / (1.0 + np.exp(-logits))
    sc = scores + bias[None, :]
    g = sc.reshape(-1, N_GROUP, E // N_GROUP)
    group_scores = np.sort(g, axis=-1)[..., -2:].sum(-1)
    gidx = np.argsort(-group_scores, axis=-1, kind="stable")[:, :TOPK_GROUP]
    gmask = np.zeros((x.shape[0], N_GROUP), np.bool_)
    np.put_along_axis(gmask, gidx, True, axis=-1)
    emask = np.repeat(gmask, E // N_GROUP, axis=-1)
    masked = np.where(emask, sc, -np.inf)
    topk_idx = np.argsort(-masked, axis=-1, kind="stable")[:, :TOPK]
    w = np.take_along_axis(scores, topk_idx, axis=-1)
    w = w / (w.sum(-1, keepdims=True) + 1e-20)
    return topk_idx, w


def _plan(topk_idx, topk_w):
    flat_e = topk_idx.ravel()
    flat_t = np.repeat(np.arange(topk_idx.shape[0]), TOPK)
    flat_w = (topk_w * ROUTED_SCALE).ravel().astype(np.float32)
    order = np.argsort(flat_e, kind="stable")
    sorted_t = flat_t[order]
    sorted_w = flat_w[order]
    counts = np.bincount(flat_e, minlength=E)
    offsets = np.concatenate([[0], np.cumsum(counts)])

    two_slots, one_slots = [], []
    for e in range(E):
        toks = sorted_t[offsets[e]:offsets[e + 1]]
        ws_ = sorted_w[offsets[e]:offsets[e + 1]]
        n = len(toks)
        if n == 0:
            continue
        pos = 0
        while n - pos > 128:
            two_slots.append((e, toks[pos:pos + 256], ws_[pos:pos + 256]))
            pos += 256
        if n - pos > 0:
            one_slots.append((e, toks[pos:], ws_[pos:]))

    best = None
    for a in range(9):
        for b in range(9):
            for e2 in range(8):
                for e1 in range(8):
                    if a > len(one_slots) or b > len(two_slots):
                        continue
                    t2 = len(two_slots) + a - b + e2
                    t1 = len(one_slots) - a + 2 * b + e1
                    if t2 % NCORES or t1 % NCORES or t2 + t1 == 0:
                        continue
                    cost = 3 * (a + 2 * e2 + e1) + 2 * (b + e2 + e1)
                    if best is None or cost < best[0]:
                        best = (cost, a, b, e2, e1)
    _, a, b, e2, e1 = best
    for _ in range(a):
        one_slots.sort(key=lambda s: len(s[1]))
        two_slots.append(one_slots.pop(0))
    for _ in range(b):
        two_slots.sort(key=lambda s: len(s[1]))
        e, tk, ws_ = two_slots.pop()
        one_slots.append((e, tk[:128], ws_[:128]))
        one_slots.append((e, tk[128:], ws_[128:]))
    empty = (0, np.empty(0, np.int64), np.empty(0, np.float32))
    for _ in range(e2):
        two_slots.append(empty)
    for _ in range(e1):
        one_slots.append(empty)

    n2 = len(two_slots) // NCORES
    n1 = len(one_slots) // NCORES
    per_core = [[] for _ in range(NCORES)]
    for si, s in enumerate(two_slots):
        per_core[si % NCORES].append(s)
    for si, s in enumerate(one_slots):
        per_core[si % NCORES].append(s)
    return per_core, n2, n1


def _pack_k(a):
    m = a.shape[1]
    return np.ascontiguousarray(
        a.reshape(KT, 128, m).transpose(1, 0, 2).reshape(128, KT * m))


def _pack_w_chunks(w):
    """[D, I] -> [128, 16*I]: 4-ktile chunks side by side."""
    return np.ascontiguousarray(
        w.reshape(4, 4, 128, I).transpose(2, 0, 1, 3).reshape(128, 16 * I))


def build_in_maps(inputs):
    x = np.asarray(inputs["hidden_states"], np.float32)
    gate_w = np.asarray(inputs["gate_w"], np.float32)
    bias = np.asarray(inputs["e_score_correction_bias"], np.float32)
    w_gate = np.asarray(inputs["w_gate"], np.float32)
    w_up = np.asarray(inputs["w_up"], np.float32)
    w_down = np.asarray(inputs["w_down"], np.float32)
    ws_gate = np.asarray(inputs["ws_gate"], np.float32)
    ws_up = np.asarray(inputs["ws_up"], np.float32)
    ws_down = np.asarray(inputs["ws_down"], np.float32)

    topk_idx, topk_w = _route_numpy(x, gate_w, bias)
    per_core, n2, n1 = _plan(topk_idx, topk_w)
    slot_caps = [256] * n2 + [128] * n1
    off = _col_layout(slot_caps)

    xt_bf = np.ascontiguousarray(x.T.astype(BF))
    xt_packed = _pack_k(xt_bf)
    wg_bf = w_gate.astype(BF)
    wu_bf = w_up.astype(BF)
    wd_bf = w_down.astype(BF)
    wcache = {}

    def expert_w(e):
        if e not in wcache:
            wcache[e] = np.concatenate([
                _pack_w_chunks(wg_bf[e]),
                _pack_w_chunks(wu_bf[e]),
                wd_bf[e].reshape(IT, 128, D).transpose(1, 0, 2)
                .reshape(128, IT * D)], axis=1)
        return wcache[e]

    wsg_bf = ws_gate.astype(BF)
    wsu_bf = ws_up.astype(BF)
    wsd_bf = ws_down.astype(BF)
    identity = np.eye(128, dtype=BF)

    in_maps = []
    for c in range(NCORES):
        wsd_sl = np.zeros((ISC * 128, D), BF)
        wsd_sl[:ISH] = wsd_bf[c * ISH:(c + 1) * ISH]
        parts = [xt_packed]
        for s, (e, idx, _) in enumerate(per_core[c]):
            cap = slot_caps[s]
            xg = np.zeros((D, cap), BF)
            if len(idx):
                xg[:, :len(idx)] = xt_bf[:, idx]
            parts.append(_pack_k(xg))
        parts.append(identity)
        parts.append(_pack_k(wsg_bf[:, c * ISH:(c + 1) * ISH]))
        parts.append(_pack_k(wsu_bf[:, c * ISH:(c + 1) * ISH]))
        parts.append(wsd_sl.reshape(ISC, 128, D).transpose(1, 0, 2)
                     .reshape(128, ISC * D))
        for s, (e, idx, _) in enumerate(per_core[c]):
            parts.append(expert_w(e))
        din = np.ascontiguousarray(np.concatenate(parts, axis=1))
        assert din.shape[1] == off["total"]
        in_maps.append({"din": din})
    return in_maps, per_core, n2, n1


def kernel(**inputs):
    in_maps, per_core, n2, n1 = build_in_maps(inputs)
    nc = get_program(n2, n1)
    res = run_bass_kernel_spmd(nc, in_maps, core_ids=list(range(NCORES)))

    slot_caps = [256] * n2 + [128] * n1
    out = np.zeros((T, D), np.float32)
    for c in range(NCORES):
        r = res.results[c]["yo"].astype(np.float32)
        out += r[:8].reshape(T, D)
        row = 8
        for s, (e, idx, wv) in enumerate(per_core[c]):
            cap = slot_caps[s]
            ntt = cap // 128
            y = r[row:row + ntt].reshape(cap, D)
            row += ntt
            if len(idx):
                out[idx] += wv[:, None] * y[:len(idx)]
    return out.astype(np.float32)
